# revision 10
# baseline (speedup 1.0000x reference)
"""Trainium2 Bass kernel for BiasedAxialAttention (tied row attention), 8-core SPMD.

Math (reference, in the transposed frame pairT[a,b,:] = pair[0,b,a,:]):
    x  = LN(pairT);  q,k,v = x@Wq,Wk,Wv (scaled);  b = LN(biasT)@Wb
    g  = sigmoid(x@Wg + bg)
    logits[i,j,h] = sum_{n,d} q[n,i,h,d] k[n,j,h,d] + b[i,j,h]
    attn = softmax_j(logits);  out[i,k,(h,d)] = sum_j attn[i,j,h] v[k,j,h,d]
    out = (g * out) @ Wo + bo;  final[k,i,:] = out[i,k,:]

Sharding (core c of 8):
    pr = pairT[32c:32c+32]              -> q,k,v partials over the tied axis n
    pc = pairT[:,32c:32c+32] (k-major)  -> gate for the core's output columns
    br = biasT[32c:32c+32]              -> b slice (AllGathered, bf16)
    partial logits AllReduce (bf16, 512KB/half); each core then computes
    out[:, k_slice] which the host transposes/concatenates.

All matmul operands are bf16 (fp32 PSUM accumulation); LN statistics and
softmax accumulators stay fp32.  The softmax normalization (1/Z) is folded
into the attention transpose by streaming diag(1/Z) through the PE instead
of the identity.  LN scale/shift is applied via per-partition scale+bias
ops ahead of the PE transpose.  LN biases must be zero and the mask
all-ones (asserted; true for this module's setup_inputs).
"""
import os
import sys

for _p in ("/opt/trn_rl_repo", "/root/.axon_site/_ro/trn_rl_repo"):
    if os.path.isdir(_p) and _p not in sys.path:
        sys.path.append(_p)

import math
import numpy as np

N_CORES = 8
L = 256
D = 128
H = 8
DH = 32
HD = H * DH          # 256
NS = L // N_CORES    # 32
NTOK = NS * L        # 8192
NT = NTOK // 128     # 64 token tiles
GRP = 8              # token tiles per LN group
NG = NT // GRP       # 8 LN groups
EPS = 1e-5

_cache = {}


def _build():
    if "nc" in _cache:
        return _cache["nc"]
    from contextlib import ExitStack

    import concourse.bacc as bacc
    import concourse.tile as tile
    from concourse import mybir

    F32 = mybir.dt.float32
    BF = mybir.dt.bfloat16
    AF = mybir.ActivationFunctionType
    ALU = mybir.AluOpType

    nc = bacc.Bacc("TRN2", target_bir_lowering=False, debug=False,
                   num_devices=N_CORES)

    ei = dict(kind="ExternalInput")
    # inputs host-packed as [group, partition, j, channel] so each LN group
    # is one fully contiguous 256KB read
    pr_d = nc.dram_tensor("pr", [NG, 128, GRP, D], BF, **ei)
    pc_d = nc.dram_tensor("pc", [NG, 128, GRP, D], BF, **ei)
    br_d = nc.dram_tensor("br", [NG, 128, GRP, D], BF, **ei)
    # w_all: [D, 4*HD+128]: wq|wk|wv|wg(perm)|wb  (LN weight and scales folded)
    wall_d = nc.dram_tensor("w_all", [D, 4 * HD + 128], BF, **ei)
    # wo2: [128, 2*D]: Wo perm rows 0:128 | rows 128:256
    wo2_d = nc.dram_tensor("wo2", [128, 2 * D], BF, **ei)
    bo_d = nc.dram_tensor("bo", [D, 1], F32, **ei)
    bg_d = nc.dram_tensor("bg2", [128, 2], F32, **ei)  # gate bias, half-major
    id_d = nc.dram_tensor("ident", [128, 128], BF, **ei)

    out_d = nc.dram_tensor("out", [D, NTOK], BF, kind="ExternalOutput")

    with tile.TileContext(nc) as tc, ExitStack() as ctx:
        singles = ctx.enter_context(tc.tile_pool(name="singles", bufs=1))
        p2 = ctx.enter_context(tc.tile_pool(name="p2", bufs=2))
        p3 = ctx.enter_context(tc.tile_pool(name="p3", bufs=3))
        p3g = ctx.enter_context(tc.tile_pool(name="p3g", bufs=3))
        big = ctx.enter_context(tc.tile_pool(name="big", bufs=1))
        ps_t = ctx.enter_context(tc.tile_pool(name="ps_t", bufs=2, space="PSUM"))
        ps_mm = ctx.enter_context(tc.tile_pool(name="ps_mm", bufs=2, space="PSUM"))
        ps_l = ctx.enter_context(tc.tile_pool(name="ps_l", bufs=2, space="PSUM"))
        ps_e = ctx.enter_context(tc.tile_pool(name="ps_e", bufs=2, space="PSUM"))
        dram = ctx.enter_context(tc.tile_pool(name="dram", bufs=1, space="DRAM"))

        # ---------------- constants ----------------
        ident = singles.tile([128, 128], BF, tag="ident")
        nc.sync.dma_start(ident[:], id_d.ap())
        w_sb = singles.tile([128, 4 * HD + 128], BF, tag="w_sb")
        nc.sync.dma_start(w_sb[:], wall_d.ap())
        wq = w_sb[:, 0:HD]
        wk = w_sb[:, HD:2 * HD]
        wv = w_sb[:, 2 * HD:3 * HD]
        wg = w_sb[:, 3 * HD:4 * HD]
        wb = w_sb[:, 4 * HD:4 * HD + 128]
        wo_sb = singles.tile([128, 2 * D], BF, tag="wo_sb")
        nc.sync.dma_start(wo_sb[:], wo2_d.ap())
        bo_sb = singles.tile([128, 1], F32, tag="bo")
        nc.sync.dma_start(bo_sb[:], bo_d.ap())
        bg_sb = singles.tile([128, 2], F32, tag="bg")
        nc.sync.dma_start(bg_sb[:], bg_d.ap())
        eps_t = singles.tile([128, 1], F32, tag="eps")
        nc.vector.memset(eps_t[:], EPS)

        # ---------------- LN + transpose: x -> xt [D,NTOK] bf16 -------------
        def ln_transpose_gen(x_dram, xt_tile):
            for gg in range(NG):
                xb = p3g.tile([128, GRP, D], BF, tag="xbuf", bufs=3)
                nc.sync.dma_start(xb[:], x_dram.ap()[gg])
                stt = p3g.tile([128, GRP, 2], F32, tag="stats", bufs=2)
                for j in range(GRP):
                    st6 = p3.tile([128, 6], F32, tag="st6")
                    nc.vector.bn_stats(out=st6[:], in_=xb[:, j, :])
                    nc.vector.bn_aggr(out=stt[:, j, :], in_=st6[:])
                std = p3g.tile([128, GRP], F32, tag="std", bufs=2)
                nc.scalar.activation(out=std[:], in_=stt[:, :, 1],
                                     func=AF.Sqrt, bias=eps_t[:], scale=1.0)
                r_ = p3g.tile([128, GRP], F32, tag="r_", bufs=2)
                nc.vector.reciprocal(out=r_[:], in_=std[:])
                nmr = p3g.tile([128, GRP], F32, tag="nmr", bufs=2)
                nc.vector.tensor_tensor(out=nmr[:], in0=stt[:, :, 0],
                                        in1=r_[:], op=ALU.mult)
                nc.vector.tensor_scalar_mul(nmr[:], nmr[:], -1.0)
                for jp in range(GRP // 4):
                    pt = ps_t.tile([128, 4, 128], BF, tag="pt")
                    for jj in range(4):
                        j = 4 * jp + jj
                        xl = p3.tile([128, D], BF, tag="xl", bufs=4)
                        if j % 2 == 0:
                            nc.vector.tensor_scalar(
                                out=xl[:], in0=xb[:, j, :],
                                scalar1=r_[:, j:j + 1],
                                scalar2=nmr[:, j:j + 1],
                                op0=ALU.mult, op1=ALU.add)
                        else:
                            nc.scalar.activation(
                                out=xl[:], in_=xb[:, j, :], func=AF.Identity,
                                bias=nmr[:, j:j + 1], scale=r_[:, j:j + 1])
                        nc.tensor.transpose(pt[:, jj, :], xl[:], ident[:])
                    g = gg * GRP + 4 * jp
                    sl = xt_tile[:, 128 * g:128 * (g + 4)]
                    nc.vector.tensor_copy(sl, pt[:])
                yield gg

        def pump(gen, n=1):
            for _ in range(n):
                next(gen, None)

        # ---------------- P1+P2 interleaved: pr-LN || br-LN || b-proj -------
        ag_in = dram.tile([H, NTOK], BF, tag="ag_in")
        ag_out = dram.tile([N_CORES, H, NS, L], BF, tag="ag_out",
                           addr_space="Shared")
        xt_pr = big.tile([128, NTOK], BF, tag="xt_pr")
        xt_br = big.tile([128, NTOK], BF, tag="xt_br")
        g_pr = ln_transpose_gen(pr_d, xt_pr)
        g_br = ln_transpose_gen(br_d, xt_br)
        for gg in range(NG):
            pump(g_pr)
            pump(g_br)
            for t5 in (2 * gg, 2 * gg + 1):
                pb = ps_mm.tile([128, 512], F32, tag="proj")
                nc.tensor.matmul(pb[:], wb,
                                 xt_br[:, 512 * t5:512 * (t5 + 1)],
                                 start=True, stop=True)
                bev = p3.tile([H, 512], BF, tag="b_ev", bufs=2)
                nc.vector.tensor_copy(bev[:], pb[:H, :])
                nc.gpsimd.dma_start(ag_in[:, 512 * t5:512 * (t5 + 1)], bev[:])
        nc.gpsimd.collective_compute(
            "AllGather", ALU.bypass,
            replica_groups=[list(range(N_CORES))],
            ins=[ag_in.opt()], outs=[ag_out.opt()],
        )
        xt_pc = big.tile([128, NTOK], BF, tag="xt_pc")
        g_pc = ln_transpose_gen(pc_d, xt_pc)

        # ---------------- P3/P4: q,k proj -> n-block pack -> logits ---------
        # pack tile pk[(d,nl), g, i]: chunk g holds tied rows n in [4g,4g+4),
        # nl = n-4g; contraction (n,d) split into 8 chunks of 128.
        ar_in = [dram.tile([4, 2, 128, L], BF, tag=f"ar_in{i}",
                           name=f"ar_in{i}") for i in range(2)]
        ar_out = [dram.tile([4, 2, 128, L], BF, tag=f"ar_out{i}",
                            name=f"ar_out{i}", addr_space="Shared")
                  for i in range(2)]
        pack_tiles = [big.tile([128, GRP, L], BF, tag=f"pk{i}",
                               name=f"pk{i}")
                      for i in range(8)]

        for half in range(2):
            packs = {}
            for ti, w_ in ((0, wq), (1, wk)):
                stg = big.tile([128, NTOK], BF, tag="st3")
                for t5 in range(16):
                    pp = ps_mm.tile([128, 512], F32, tag="proj")
                    nc.tensor.matmul(pp[:], w_[:, 128 * half:128 * (half + 1)],
                                     xt_pr[:, 512 * t5:512 * (t5 + 1)],
                                     start=True, stop=True)
                    sl = stg[:, 512 * t5:512 * (t5 + 1)]
                    nc.vector.tensor_copy(sl, pp[:])
                # pack: [(hq,d), (g,nl,i)] -> [(nl,d), (g,i)] per head
                for hq in range(4):
                    h = 4 * half + hq
                    pk_ = pack_tiles[4 * ti + hq]
                    packs.setdefault(h, [None, None])[ti] = pk_
                    srcv = stg[32 * hq:32 * (hq + 1), :].rearrange(
                        "d (g nl i) -> d nl g i", nl=4, i=L)
                    for nl in range(4):
                        eng = nc.sync if (nl % 2 == 0) else nc.scalar
                        eng.dma_start(pk_[32 * nl:32 * (nl + 1), :, :],
                                      srcv[:, nl, :, :])
            for hq in range(4):
                h = 4 * half + hq
                pq, pk = packs[h]
                for ih in range(2):
                    pl = ps_l.tile([128, L], F32, tag="logit")
                    for g in range(8):
                        nc.tensor.matmul(
                            pl[:], pq[:, g, 128 * ih:128 * (ih + 1)],
                            pk[:, g, :], start=(g == 0), stop=(g == 7))
                    ls = p2.tile([128, L], BF, tag="l_ev")
                    nc.vector.tensor_copy(ls[:], pl[:])
                    nc.sync.dma_start(ar_in[half][hq, ih, :, :], ls[:])
            nc.gpsimd.collective_compute(
                "AllReduce", ALU.add,
                replica_groups=[list(range(N_CORES))],
                ins=[ar_in[half].opt()], outs=[ar_out[half].opt()],
            )

        pump(g_pc, NG)

        # ---------------- P5: v-proj -> vbig [j, jh, kg, h, kl, d] ----------
        # per (jh,kg,h) the (kl,d) block is contiguous -> legal 1-free-dim
        # stationary operand for the einsum matmuls
        vbig = big.tile([128, 2, 8, H, 4, DH], BF, tag="vbig")
        for t in range(NT):
            k, jh = t // 2, t % 2
            pv = ps_mm.tile([128, 512], F32, tag="proj")
            nc.tensor.matmul(pv[:, :HD],
                             xt_pr[:, 128 * t:128 * (t + 1)],
                             wv, start=True, stop=True)
            dst = vbig[:, jh, k // 4, :, k % 4, :]
            if t % 2 == 0:
                nc.vector.tensor_copy(dst, pv[:, :HD])
            else:
                nc.scalar.activation(out=dst, in_=pv[:, :HD],
                                     func=AF.Copy)

        # ---------------- softmax (bias add + exp + 1/Z via diag transpose) -
        at_t = big.tile([128, 2, H, L], BF, tag="at_t")

        def softmax(h):
            half, hq = h // 4, h % 4
            lsb = p2.tile([128, 2, L], BF, tag="sm_l")
            nc.gpsimd.dma_start(
                lsb[:], ar_out[half][hq].rearrange("ih p j -> p ih j"))
            for ih in range(2):
                bt = p2.tile([128, L], BF, tag="sm_b")
                nc.gpsimd.dma_start(bt[:], ag_out[4 * ih:4 * (ih + 1), h, :, :])
                ls = lsb[:, ih, :]
                nc.gpsimd.tensor_tensor(out=ls, in0=ls, in1=bt[:], op=ALU.add)
                nmx = p3.tile([128, 1], F32, tag="sm_m")
                nc.vector.tensor_reduce(out=nmx[:], in_=ls,
                                        axis=mybir.AxisListType.X,
                                        op=ALU.max, negate=True)
                pe_ = p2.tile([128, L], BF, tag="sm_e")
                sme = p3.tile([128, 1], F32, tag="sm_s")
                nc.scalar.activation(out=pe_[:], in_=ls, func=AF.Exp,
                                     bias=nmx[:], scale=1.0, accum_out=sme[:])
                rs = p3.tile([128, 1], F32, tag="sm_r")
                nc.vector.reciprocal(out=rs[:], in_=sme[:])
                dg = p2.tile([128, 128], BF, tag="sm_d")
                nc.vector.tensor_scalar(out=dg[:], in0=ident[:],
                                        scalar1=rs[:], scalar2=None,
                                        op0=ALU.mult)
                pt2 = ps_t.tile([128, 2, 128], F32, tag="pt")
                for jh in range(2):
                    nc.tensor.matmul(pt2[:, jh, :],
                                     pe_[:, 128 * jh:128 * (jh + 1)],
                                     dg[:], start=True, stop=True)
                dst = at_t[:, :, h, 128 * ih:128 * (ih + 1)]
                if ih == 0:
                    nc.vector.tensor_copy(dst, pt2[:])
                else:
                    nc.scalar.activation(
                        out=dst, in_=pt2[:].rearrange("p a b -> p (a b)"),
                        func=AF.Copy)

        def ein_mms(kg, h, ein_t):
            po = ps_e.tile([128, L], F32, tag="ein")
            for jh in range(2):
                lhs = vbig[:, jh, kg, h, :, :].rearrange("p a b -> p (a b)")
                nc.tensor.matmul(po[:], lhs, at_t[:, jh, h, :],
                                 start=(jh == 0), stop=(jh == 1))
            if h % 2 == 0:
                nc.vector.tensor_copy(ein_t[:, h, :], po[:])
            else:
                nc.scalar.activation(out=ein_t[:, h, :], in_=po[:],
                                     func=AF.Copy)

        # ---------------- phase A: gates, softmax h0-3, ein h0-3 ------------
        # gate tensors reuse the st3 / xt_br buffers (dead by now)
        gs1 = big.tile([128, NTOK], BF, tag="st3", name="gs1")
        gs2 = big.tile([128, NTOK], BF, tag="xt_br", name="gs2")

        def gs_view(kg, q):
            t_ = (gs1 if kg < 4 else gs2)
            return t_[:].rearrange("p (kg q n) -> p kg q n", kg=4, q=2)[
                :, kg % 4, q, :]

        for kg in range(8):
            for q in range(2):
                for cc in range(2):
                    pg = ps_mm.tile([128, 512], F32, tag="proj")
                    nc.tensor.matmul(
                        pg[:], wg[:, 128 * q:128 * (q + 1)],
                        xt_pc[:, 1024 * kg + 512 * cc:
                              1024 * kg + 512 * (cc + 1)],
                        start=True, stop=True)
                    gv = gs_view(kg, q)[:, 512 * cc:512 * (cc + 1)]
                    nc.scalar.activation(out=gv, in_=pg[:], func=AF.Sigmoid,
                                         bias=bg_sb[:, q:q + 1], scale=1.0)

        ein_tiles = [big.tile([128, H, L], BF, tag=f"pk{kg}",
                              name=f"ein{kg}") for kg in range(8)]
        for h in range(4):
            softmax(h)
            for kg in range(8):
                ein_mms(kg, h, ein_tiles[kg])

        # ---------------- phase B: softmax h4-7, ein h4-7, okg, gate, Wo ----
        for h in range(4, 8):
            softmax(h)
            for kg in range(8):
                ein_mms(kg, h, ein_tiles[kg])
        for kg in range(8):
            ein_t = ein_tiles[kg]
            okg = big.tile([128, 2, 4, L], BF, tag="okg", bufs=2)
            for kl in range(4):
                for q in range(2):
                    eng = (nc.sync, nc.scalar, nc.gpsimd)[(2 * kl + q) % 3]
                    eng.dma_start(
                        okg[:, q, kl, :],
                        ein_t[32 * kl:32 * (kl + 1), 4 * q:4 * (q + 1), :])
            for q in range(2):
                ov = okg[:, q, :, :].rearrange("p kl i -> p (kl i)")
                nc.vector.tensor_tensor(out=ov, in0=ov, in1=gs_view(kg, q),
                                        op=ALU.mult)
            for cc in range(2):
                pw = ps_l.tile([128, 512], F32, tag="logit")
                m0 = okg[:, 0, :, :].rearrange("p kl i -> p (kl i)")
                m1 = okg[:, 1, :, :].rearrange("p kl i -> p (kl i)")
                nc.tensor.matmul(pw[:], wo_sb[:, 0:D],
                                 m0[:, 512 * cc:512 * (cc + 1)],
                                 start=True, stop=False)
                nc.tensor.matmul(pw[:], wo_sb[:, D:2 * D],
                                 m1[:, 512 * cc:512 * (cc + 1)],
                                 start=False, stop=True)
                fo = p2.tile([128, 512], BF, tag="fo")
                nc.scalar.activation(out=fo[:], in_=pw[:], func=AF.Identity,
                                     bias=bo_sb[:], scale=1.0)
                nc.sync.dma_start(
                    out_d.ap()[:, 1024 * kg + 512 * cc:
                               1024 * kg + 512 * (cc + 1)],
                    fo[:])

    nc.compile()
    _cache["nc"] = nc
    return nc


def _prep_inputs(inputs):
    import ml_dtypes

    BF = ml_dtypes.bfloat16
    pair = np.asarray(inputs["pair"], dtype=np.float32)
    bias = np.asarray(inputs["bias"], dtype=np.float32)
    mask = np.asarray(inputs["mask"])
    assert bool(mask.all()), "kernel specialized for all-ones mask"
    lnpw = np.asarray(inputs["ln_pair_w"], np.float32)
    lnpb = np.asarray(inputs["ln_pair_b"], np.float32)
    lnbw = np.asarray(inputs["ln_bias_w"], np.float32)
    lnbb = np.asarray(inputs["ln_bias_b"], np.float32)
    assert np.abs(lnpb).max() == 0.0 and np.abs(lnbb).max() == 0.0, \
        "kernel specialized for zero LN biases"
    Wq = np.asarray(inputs["Wq"], np.float32)
    Wk = np.asarray(inputs["Wk"], np.float32)
    Wv = np.asarray(inputs["Wv"], np.float32)
    Wb = np.asarray(inputs["Wb"], np.float32)
    Wg = np.asarray(inputs["Wg"], np.float32)
    bg = np.asarray(inputs["bg"], np.float32)
    Wo = np.asarray(inputs["Wo"], np.float32)
    bo = np.asarray(inputs["bo"], np.float32)

    pairT = np.ascontiguousarray(pair[0].transpose(1, 0, 2))
    biasT = np.ascontiguousarray(bias[0].transpose(1, 0, 2))

    wb_pad = np.zeros((D, 128), np.float32)
    wb_pad[:, :H] = lnbw[:, None] * Wb
    # permutation for the einsum output partitions: P = d*4 + hq (per half)
    perm = np.empty(HD, np.int64)
    for half in range(2):
        for hq in range(4):
            for d_ in range(DH):
                perm[half * 128 + d_ * 4 + hq] = (4 * half + hq) * DH + d_
    wg_s = lnpw[:, None] * Wg
    wg_perm = wg_s[:, perm]
    w_all = np.concatenate([
        (lnpw[:, None] * Wq) / math.sqrt(DH),
        (lnpw[:, None] * Wk) / math.sqrt(L),
        lnpw[:, None] * Wv,
        wg_perm,
        wb_pad,
    ], axis=1)
    wo_p = Wo[perm, :]
    wo2 = np.concatenate([wo_p[0:128, :], wo_p[128:256, :]], axis=1)
    bg_perm = bg[perm]

    def grp4(tok_major):  # [NTOK, D] f32 -> [NG, 128, GRP, D] bf16
        return np.ascontiguousarray(
            tok_major.reshape(NG, GRP, 128, D).transpose(0, 2, 1, 3)
        ).astype(BF)

    base = {
        "w_all": np.ascontiguousarray(w_all).astype(BF),
        "wo2": np.ascontiguousarray(wo2).astype(BF),
        "bo": bo.reshape(D, 1).copy(),
        "bg2": np.ascontiguousarray(bg_perm.reshape(2, 128).T),
        "ident": np.eye(128, dtype=np.float32).astype(BF),
    }
    in_maps = []
    for c in range(N_CORES):
        sl = slice(NS * c, NS * (c + 1))
        m = dict(base)
        m["pr"] = grp4(pairT[sl].reshape(NTOK, D))
        m["pc"] = grp4(pairT[:, sl].transpose(1, 0, 2).reshape(NTOK, D))
        m["br"] = grp4(biasT[sl].reshape(NTOK, D))
        in_maps.append(m)
    return in_maps


def _sharded_fn(nc):
    """Build (once) a cached jitted shard_map callable for the program."""
    if "fn" in _cache:
        return _cache["fn"]
    import jax
    import numpy as _np
    from jax.sharding import Mesh, PartitionSpec
    from jax.experimental.shard_map import shard_map
    from concourse import mybir
    from concourse import bass2jax as b2j

    b2j.install_neuronx_cc_hook()
    pid_name = (nc.partition_id_tensor.name
                if nc.partition_id_tensor is not None else None)
    in_names, out_names, out_shapes, out_dtypes = [], [], [], []
    for alloc in nc.m.functions[0].allocations:
        if not isinstance(alloc, mybir.MemoryLocationSet):
            continue
        name = alloc.memorylocations[0].name
        if alloc.kind == "ExternalInput":
            if name == pid_name:
                continue
            in_names.append(name)
        elif alloc.kind == "ExternalOutput":
            out_names.append(name)
            out_shapes.append(tuple(alloc.tensor_shape))
            out_dtypes.append(mybir.dt.np(alloc.dtype))
    n_params = len(in_names)
    n_outs = len(out_names)
    out_avals = [jax.core.ShapedArray(s, d)
                 for s, d in zip(out_shapes, out_dtypes)]
    all_names = in_names + out_names
    if pid_name is not None:
        all_names = all_names + [pid_name]

    def _body(*args):
        ops = list(args)
        if pid_name is not None:
            ops.append(b2j.partition_id_tensor())
        outs = b2j._bass_exec_p.bind(
            *ops,
            out_avals=tuple(out_avals),
            in_names=tuple(all_names),
            out_names=tuple(out_names),
            lowering_input_output_aliases=(),
            sim_require_finite=True,
            sim_require_nnan=True,
            nc=nc,
        )
        return tuple(outs)

    devices = jax.devices()[:N_CORES]
    mesh = Mesh(_np.asarray(devices), ("core",))
    in_specs = (PartitionSpec("core"),) * (n_params + n_outs)
    out_specs = (PartitionSpec("core"),) * n_outs
    donate = tuple(range(n_params, n_params + n_outs))
    fn = jax.jit(
        shard_map(_body, mesh=mesh, in_specs=in_specs, out_specs=out_specs,
                  check_rep=False),
        donate_argnums=donate, keep_unused=True)
    _cache["fn"] = (fn, in_names, out_names, out_shapes, out_dtypes)
    return _cache["fn"]


def kernel(**inputs):
    nc = _build()
    in_maps = _prep_inputs(inputs)
    fn, in_names, out_names, out_shapes, out_dtypes = _sharded_fn(nc)
    concat_in = [np.concatenate([in_maps[c][n] for c in range(N_CORES)], axis=0)
                 for n in in_names]
    concat_zeros = [np.zeros((N_CORES * s[0], *s[1:]), d)
                    for s, d in zip(out_shapes, out_dtypes)]
    out_arrs = fn(*concat_in, *concat_zeros)
    oc_all = np.asarray(out_arrs[out_names.index("out")]) \
        .astype(np.float32).reshape(N_CORES, D, NTOK)
    out = np.empty((1, L, L, D), dtype=np.float32)
    for c in range(N_CORES):
        out[0, NS * c:NS * (c + 1)] = \
            oc_all[c].reshape(D, NS, L).transpose(1, 2, 0)
    return out


if __name__ == "__main__":
    _build()
    print("build ok")
